# revision 3
# baseline (speedup 1.0000x reference)
"""Trainium2 Bass kernel for nn_Classifier_42588895707508 (uniform 16-chunk).

Computation (see reference):
    pool_k[b, h] = max_{s < eff_k[b]} x_k[b, s, h]      (k = 1, 2)
    out[b, c]    = sum_h pool_1[b,h] W[c,h] + pool_2[b,h] W[c, 768+h] + b[c]

Design (memory regime, fp16):
  * Valid prefixes only, packed fp16, h%128 on partitions; each
    (kind, sample) row cut into uniform 16-wide column chunks along the
    seq dim (last chunk -inf-padded).  Slots sorted by chunk count desc
    so each layer's live slots form a prefix; pack order = (kind, layer,
    slot) -> every DMA section and fold op is a uniform dense block.
  * DMA lands straight into a ring of stage tiles [128, rows, 16]
    (no intermediate data tiles).  Per section: 3 tensor_tensor max
    folds (16->8->4->2), all in DVE 2x mode, last writing a persistent
    pool tile pd2[128, chunkrows, 2].
  * Layer combine: pool rows of layer l form a contiguous range aligned
    with the slot prefix; one width-2 max op per layer folds everything
    into the layer-0 block.  One final 2->1 op per kind (384 outs).
  * Epilogue: 6 accumulating matmuls per kind (K=128, fp16, PSUM f32),
    PSUM -> SBUF copy, DMA out.  Host adds bias and un-permutes.
"""

import numpy as np

B, S, H, C = 512, 256, 768, 2
NCORES = 8
CH = H // 128
KINDS = 2
SLOTS = B // NCORES
NEG = np.float16(-60000.0)

CW = 16                       # chunk width (cols)
CCOLS = CH * CW               # pack cols per chunk (96)
SEC_MAX = 128                 # chunks per steady-state section
SEC_RAMP = [32, 64, 96]       # chunk budgets for the first sections
SEC_TAPER = [96, 64, 48, 32]  # chunk budgets for the last sections
STAGE_BUFS = 3                # pd8 is wide; 3 section bufs keep SBUF in budget
PW = 8                        # pool width: sections fold 16->8 only; combines
                              # and the per-kind final fold finish the tree


def _eff_lengths(m):
    am = np.argmin(np.asarray(m), axis=1)
    return np.where(am == 0, S, am).astype(np.int64)


def _plan(slot_w):
    plans = []
    for k in range(KINDS):
        w = np.asarray(slot_w[k], dtype=np.int64)
        nch = (w + CW - 1) // CW
        order = np.argsort(-nch, kind="stable")
        nch_s = nch[order]
        lmax = int(nch_s.max())
        c = [int(np.sum(nch_s > l)) for l in range(lmax)]
        plans.append(dict(order=order, nch=nch_s, c=c, lmax=lmax))
    return plans


def _layout(plans):
    """Chunk order: (kind, layer, slot). Returns sections + layer bases."""
    layer_base = {}
    nchunks = 0
    for k, p in enumerate(plans):
        for l in range(p["lmax"]):
            layer_base[(k, l)] = nchunks
            nchunks += p["c"][l]
    # sections (chunk ranges): ramp up, steady, taper down
    taper_total = sum(SEC_TAPER)
    secs = []
    pos = 0
    i = 0
    while pos < nchunks:
        rem = nchunks - pos
        if i < len(SEC_RAMP) and rem > SEC_RAMP[i] + taper_total:
            cap = SEC_RAMP[i]
        elif rem > SEC_MAX + taper_total:
            cap = SEC_MAX
        elif rem > taper_total:
            cap = rem - taper_total
        else:
            # drain the taper list proportionally to what's left
            cap = max(24, rem // 3 + (rem % 3 > 0))
        take = min(cap, rem)
        secs.append((pos, pos + take))
        pos += take
        i += 1
    return nchunks, layer_base, secs


def _build_program(nchunks, layer_base, secs, plans):
    import concourse.bacc as bacc
    import concourse.mybir as mybir
    from concourse.tile import TileContext

    f16 = mybir.dt.float16
    f32 = mybir.dt.float32
    MAX = mybir.AluOpType.max

    nc = bacc.Bacc("TRN2", target_bir_lowering=False, debug=False,
                   num_devices=NCORES)
    R = nchunks * CCOLS
    p_in = nc.dram_tensor("p", [128, R], f16, kind="ExternalInput")
    wt_in = nc.dram_tensor("wt", [128, KINDS * CH, C], f16, kind="ExternalInput")
    out_d = nc.dram_tensor("out", [C, KINDS * SLOTS], f32, kind="ExternalOutput")

    # trigger section for each (kind, layer) combine: section containing the
    # layer's last chunk, and layer-0 of the kind must be complete too
    def sec_of(chunk):
        for si, (lo, hi) in enumerate(secs):
            if lo <= chunk < hi:
                return si
        raise ValueError(chunk)

    comb_at = {}
    final_at = [0, 0]
    for k, p in enumerate(plans):
        s0 = sec_of(layer_base[(k, 0)] + p["c"][0] - 1)
        for l in range(1, p["lmax"]):
            sl = sec_of(layer_base[(k, l)] + p["c"][l] - 1)
            comb_at.setdefault(max(s0, sl), []).append((k, l))
            final_at[k] = max(final_at[k], max(s0, sl))
        final_at[k] = max(final_at[k], s0)

    with TileContext(nc) as tc:
        with (
            tc.tile_pool(name="stage", bufs=STAGE_BUFS) as stage_pool,
            tc.tile_pool(name="small", bufs=1) as small_pool,
            tc.tile_pool(name="psum", bufs=1, space="PSUM") as psum_pool,
        ):
            wt_t = small_pool.tile([128, KINDS * CH, C], f16, tag="wt")
            pd8 = small_pool.tile([128, nchunks * CH, PW], f16, tag="pd8")
            a1 = small_pool.tile([128, KINDS, SLOTS * CH], f16, tag="a1")
            out_sb = small_pool.tile([C, KINDS * SLOTS], f32, tag="osb")

            def combine(k, l):
                p = plans[k]
                n = p["c"][l] * CH
                a0 = layer_base[(k, 0)] * CH
                lo = layer_base[(k, l)] * CH
                nc.vector.tensor_tensor(
                    out=pd8[:, a0 : a0 + n, :],
                    in0=pd8[:, a0 : a0 + n, :],
                    in1=pd8[:, lo : lo + n, :],
                    op=MAX,
                )

            def final_and_epilogue(k):
                a0 = layer_base[(k, 0)] * CH
                A = pd8[:, a0 : a0 + SLOTS * CH, :]
                nc.vector.tensor_tensor(
                    out=A[:, :, :4], in0=A[:, :, :4], in1=A[:, :, 4:8], op=MAX,
                )
                nc.vector.tensor_tensor(
                    out=A[:, :, :2], in0=A[:, :, :2], in1=A[:, :, 2:4], op=MAX,
                )
                nc.vector.tensor_tensor(
                    out=a1[:, k, :].rearrange("p (r o) -> p r o", o=1),
                    in0=A[:, :, 0:1],
                    in1=A[:, :, 1:2],
                    op=MAX,
                )
                ps = psum_pool.tile([C, SLOTS], f32, tag=f"ps{k}", name=f"ps{k}")
                arr = a1[:, k, :].rearrange("p (s c) -> p s c", c=CH)
                for ch in range(CH):
                    nc.tensor.matmul(
                        ps[:, :],
                        lhsT=wt_t[:, k * CH + ch, :],
                        rhs=arr[:, :, ch],
                        start=(ch == 0),
                        stop=(ch == CH - 1),
                    )
                nc.scalar.copy(
                    out=out_sb[:, k * SLOTS : (k + 1) * SLOTS], in_=ps[:, :]
                )
                nc.sync.dma_start(
                    out=out_d[:, k * SLOTS : (k + 1) * SLOTS],
                    in_=out_sb[:, k * SLOTS : (k + 1) * SLOTS],
                )

            for si, (lo, hi) in enumerate(secs):
                nch = hi - lo
                rows = nch * CH
                st = stage_pool.tile([128, SEC_MAX * CH, CW], f16, tag="stage")
                sec = st[:, :rows, :]
                nc.sync.dma_start(
                    out=sec.rearrange("p r w -> p (r w)"),
                    in_=p_in[:, lo * CCOLS : hi * CCOLS],
                )
                if si == 1:
                    nc.scalar.dma_start(out=wt_t, in_=wt_in[:, :, :])
                nc.vector.tensor_tensor(
                    out=pd8[:, lo * CH : hi * CH, :],
                    in0=sec[:, :, :8], in1=sec[:, :, 8:16],
                    op=MAX,
                )
                for (k, l) in comb_at.get(si, []):
                    combine(k, l)
                for k in range(KINDS):
                    if final_at[k] == si:
                        final_and_epilogue(k)

    nc.compile()
    return nc


_NC_CACHE = {}


def kernel(x1, x2, m1, m2, W, b, _run_opts=None):
    from concourse.bass_utils import run_bass_kernel_spmd

    x1 = np.asarray(x1)
    x2 = np.asarray(x2)
    W32 = np.asarray(W, dtype=np.float32)
    b32 = np.asarray(b, dtype=np.float32)
    effs = [_eff_lengths(m1), _eff_lengths(m2)]
    orders = [np.argsort(-effs[k], kind="stable") for k in range(KINDS)]
    slot_w = [effs[k][orders[k][::NCORES]].astype(np.int64) for k in range(KINDS)]

    plans = _plan(slot_w)
    nchunks, layer_base, secs = _layout(plans)

    key = (nchunks, tuple(sorted(layer_base.items())), tuple(secs))
    nc = _NC_CACHE.get(key)
    if nc is None:
        nc = _build_program(nchunks, layer_base, secs, plans)
        _NC_CACHE[key] = nc

    R = nchunks * CCOLS
    xh = [x1.astype(np.float16), x2.astype(np.float16)]
    packs = np.full((NCORES, 128, R), NEG, dtype=np.float16)
    for k in range(KINDS):
        p = plans[k]
        eff, order = effs[k], orders[k]
        xk = xh[k]
        for l in range(p["lmax"]):
            cb = layer_base[(k, l)]
            e_lo = l * CW
            for sp in range(p["c"][l]):
                rank = p["order"][sp]
                col0 = (cb + sp) * CCOLS
                for c in range(NCORES):
                    bidx = order[rank * NCORES + c]
                    e = int(eff[bidx])
                    n = min(CW, max(0, e - e_lo))
                    if n <= 0:
                        continue
                    dst = packs[c][:, col0 : col0 + CCOLS].reshape(128, CH, CW)
                    dst[:, :, :n] = (
                        xk[bidx, e_lo : e_lo + n, :]
                        .reshape(n, CH, 128)
                        .transpose(2, 1, 0)
                    )

    wtp = np.ascontiguousarray(
        W32.astype(np.float16).reshape(C, KINDS, CH, 128).transpose(3, 1, 2, 0)
    ).reshape(128, KINDS * CH, C)

    in_maps = [{"p": packs[c], "wt": wtp} for c in range(NCORES)]

    res = None
    last_err = None
    for _attempt in range(3):
        try:
            res = run_bass_kernel_spmd(
                nc, in_maps, core_ids=list(range(NCORES)), **(_run_opts or {})
            )
            break
        except Exception as e:
            last_err = e
    if res is None:
        raise last_err

    out_full = np.zeros((B, C), dtype=np.float32)
    res_all = np.stack([res.results[c]["out"] for c in range(NCORES)])
    for k in range(KINDS):
        p = plans[k]
        part = res_all[:, :, k * SLOTS : (k + 1) * SLOTS]  # [core, C, sorted pos]
        inv = np.empty(SLOTS, dtype=np.int64)
        inv[p["order"]] = np.arange(SLOTS)
        part_rank = part[:, :, inv]                        # [core, C, rank]
        pr = part_rank.transpose(2, 0, 1).reshape(B, C)
        out_full[orders[k]] += pr
    out_full += b32[None, :]
    if _run_opts is not None:
        kernel._last_res = res
    return out_full
